# revision 12
# baseline (speedup 1.0000x reference)
"""Trainium2 Bass kernel for nn_Lowpass: 2D DCT -> keep 15x15 low-freq block -> 2D IDCT.

The op collapses to out[b,c] = P @ x[b,c] @ P^T with P = Di[:, :15] @ D[:15, :]
(a fixed 32x32 projection), data-parallel over 8 NeuronCores (3072 images each).

Measured-cost-driven design ("two-phase"). Key facts measured on this HW:
  - strided DMA (128B elements) runs ~full rate ALONE (~1.1us / 0.5MB) but
    any two concurrent DMA streams where one is strided collapse to ~3-4x
    worse aggregate. Contiguous+contiguous coexist fine.
  - DVE stream transpose: flat src ~1.1us per [128,1024]; strided src ~2.3us.
    fp32r matmul is 4 PE cycles/row on real HW (no faster than fp32);
    bf16 with FLAT rhs is fast; any strided matmul rhs is ~2x.
So: let the DMA absorb the first shuffle (strided load puts h on partitions)
and the last shuffle (strided store from T_post's natural layout), keeping the
load window and store window DISJOINT in time (outputs parked in SBUF, 12MB).
DVE only does the two mandatory flat PSUM-evicting transposes per pack.

Per 128-image pack (image i = 4q+a, partitions grouped a in [0,4)):
  phase 1 (loads + compute, stores gated off):
    load (sync, strided):  L2[32a+h, 32q+w] = X_i[h,w]
    conv1 (ACT):           L2b = bf16(L2)
    MM1  (PE, lhsT=BD(P^T) bf16, flat rhs):  P1[32a+v, 32q+w] = (P X_i)[v,w]
    T_mid (DVE flat):      T2[32a+w, 32q+v]
    conv2 (Pool):          T2b = bf16(T2)
    MM2  (PE, flat rhs):   P2[32a+u, 32q+v] = Y_i[v,u]
    T_post (DVE flat):     S[32a+v, 32q+u] = Y_i[v,u]   (S persistent, 24 tiles)
  gate: ACT executes its stream in order; 1-element self-copies on each S tile
    are emitted after all conv1(k) [conv1(k) waits load k], so each store's
    RAW dep on its self-copy delays every store past the LAST load.
  phase 2: strided stores (scalar) mirror the load pattern.
"""

import numpy as np

N = 32
FRE = 15
NCORES = 8
IMG_TOTAL = 8192 * 3          # 24576 images of 32x32
PER_CORE = IMG_TOTAL // NCORES  # 3072
PACK = 128                    # images per pipeline iteration (0.5 MB)
NPACK = PER_CORE // PACK      # 24


def _install_tilefix():
    """This container's walrus build rejects instructions carrying >1 sem wait
    ("Too many sync wait commands" in setupSyncWait). Tile attaches all of an
    instruction's required waits to the instruction itself. Split: for any
    instruction with N>1 waits, hoist N-1 of them onto fresh same-engine nop
    instructions placed immediately before it (same blocking semantics, one
    wait per instruction). Same treatment for the kernel-tail drain."""
    from concourse import mybir, tile
    from concourse.vector_clock import ScopedClock, VectorClock

    if getattr(tile.TileContext, "_tilefix_installed", False):
        return

    orig_lower = tile.TileContext._lower_ordered_insts

    def _lower_split(self, postordered_blocks):
        nc = self.nc
        for insts in postordered_blocks.values():
            new = []
            for inst in insts:
                si = getattr(inst, "sync_info", None)
                ow = list(si.on_wait) if si is not None and si.on_wait else []
                if len(ow) > 1:
                    for w in ow[:-1]:
                        nop = mybir.InstNoOp(
                            name=nc.get_next_instruction_name(), ins=[], outs=[])
                        nop.engine = inst.engine
                        nop.sync_info = mybir.SyncInfo(
                            on_wait=[w], on_update=[])
                        new.append(nop)
                    inst.sync_info = mybir.SyncInfo(
                        on_wait=[ow[-1]], on_update=list(si.on_update))
                new.append(inst)
            insts[:] = new
        return orig_lower(self, postordered_blocks)

    def _drain_and_barrier_split(self, tick_clock, wait_clock):
        nc = self.nc
        gc = tick_clock.global_clock
        n = len(gc)
        for proc in range(n):
            t = gc[proc]
            if t <= 0:
                continue
            vec = [0] * n
            vec[proc] = t
            nop_inst = nc.sync.nop()
            wait_clock.add_sem_waits(
                nop_inst.ins, ScopedClock({None: VectorClock(vec)})
            )
        nc.sync.drain()
        nc.all_engine_barrier()
        assert self.sems is not None
        popped = nc._tile_sem_poison_stack.pop()
        assert popped is self._sem_poison
        nc.clear_and_free_semaphores(list(self.sems.allocated().values()))
        nc.all_engine_barrier()

    tile.TileContext._lower_ordered_insts = _lower_split
    tile.TileContext._drain_and_barrier = _drain_and_barrier_split
    tile.TileContext._tilefix_installed = True

    # NTFF profiling hooks don't exist in this container; make trace=True
    # degrade gracefully inside run_bass_kernel_spmd.
    import sys as _sys
    import types as _types
    if "antenv.axon_hooks" not in _sys.modules:
        m = _types.ModuleType("antenv.axon_hooks")
        m.get_axon_ntff_profile_hook = lambda: None
        _sys.modules["antenv.axon_hooks"] = m


def _p_matrix():
    i = np.arange(N)
    D = 2.0 * np.cos(np.pi * (2 * i[None, :] + 1) * i[:, None] / (2 * N))
    Di = np.linalg.inv(D)
    P = Di[:, :FRE] @ D[:FRE, :]        # float64 [32, 32]
    return P


def _bd_matrix():
    # lhsT = block-diag(P^T): BD[32A+h, 32A+v] = P[v, h]
    P = _p_matrix()
    return np.kron(np.eye(4), P.T).astype(np.float32)  # [128, 128]


def _build_program(mm_dtype_name="bfloat16", loop_reps=1, dma_only=False):
    from concourse import bass, tile
    from concourse import mybir

    F32 = mybir.dt.float32
    MMDT = getattr(mybir.dt, mm_dtype_name)
    use_conv = mm_dtype_name != "float32" and not dma_only
    FREE = PACK * 8             # 1024 free elems per [128, FREE] tile

    nc = bass.Bass("TRN2", target_bir_lowering=False, debug=False,
                   num_devices=NCORES)
    x_ext = nc.dram_tensor("x", [PER_CORE, N, N], F32, kind="ExternalInput").ap()
    p_ext = nc.dram_tensor("pconst", [128, 128], F32, kind="ExternalInput").ap()
    y_ext = nc.dram_tensor("y", [PER_CORE, N, N], F32, kind="ExternalOutput").ap()

    with tile.TileContext(nc) as tc:
        with tc.tile_pool(name="const", bufs=1) as cpool, \
             tc.tile_pool(name="xin", bufs=NPACK if dma_only else 4) as xpool, \
             tc.tile_pool(name="xb", bufs=NPACK) as xbpool, \
             tc.tile_pool(name="tok", bufs=2) as tokpool, \
             tc.tile_pool(name="t2", bufs=2) as t2pool, \
             tc.tile_pool(name="sout", bufs=NPACK) as spool, \
             tc.tile_pool(name="psA", bufs=2, space="PSUM") as papool, \
             tc.tile_pool(name="psB", bufs=2, space="PSUM") as pbpool:

            bd_f32 = cpool.tile([128, 128], F32)
            nc.sync.dma_start(bd_f32[:], p_ext[:])
            if use_conv:
                bd_mm = cpool.tile([128, 128], MMDT)
                nc.scalar.copy(bd_mm[:], bd_f32[:])
            else:
                bd_mm = bd_f32

            for rep in range(loop_reps):
                souts = []
                t2s = {}
                tok = tokpool.tile([1, NPACK], F32)
                # -------- phase 1: loads + compute, 1-pack skew -----------
                # DVE's in-order stream becomes T_mid(0), T_mid(1),
                # T_post(0), T_mid(2), T_post(1), ... so T_post(p-1) never
                # makes DVE idle-wait on MM2(p-1): PE runs MM2(p-1) while
                # DVE does T_mid(p).
                for p in range(NPACK + 1):
                    if p < NPACK:
                        base = p * PACK
                        # strided load (128B elements): h on partitions.
                        # i = 4q + a; L2[32a+h, 32q+w] = X_i[h,w]
                        L2 = xpool.tile([128, FREE], F32)
                        nc.sync.dma_start(
                            L2.rearrange("p (q w) -> p q w", w=N),
                            x_ext[base: base + PACK].rearrange(
                                "(q a) h w -> a h q w", a=4),
                        )
                        # gate token: a cheap ACT op consuming this load.
                        # ACT runs its stream in order, so anything emitted
                        # on ACT after token(NPACK-1) executes after ALL
                        # loads completed.
                        nc.scalar.copy(tok[0:1, p:p + 1], L2[0:1, 0:1])
                        if dma_only:
                            souts.append(L2)
                            continue

                        if use_conv:
                            # conv1 (ACT): bf16 copy consumed by MM1. Deep
                            # pool (bufs=NPACK) so DVE-paced MM1 consumption
                            # never back-throttles conv1 -> loads.
                            L2m = xbpool.tile([128, FREE], MMDT)
                            nc.scalar.copy(L2m[:], L2[:])
                        else:
                            L2m = L2

                        # MM1 (bf16): P1[32a+v, 32q+w] = (P X_i)[v,w]
                        P1 = papool.tile([128, FREE], F32, tag="psA")
                        for b in range(2):
                            nc.tensor.matmul(
                                P1[:, 512 * b: 512 * (b + 1)],
                                bd_mm[:, :],
                                L2m[:, 512 * b: 512 * (b + 1)],
                                start=True, stop=True,
                            )

                        # T_mid (DVE flat): T2[32a+w, 32q+v] = (P X_i)[v,w]
                        T2 = t2pool.tile([128, FREE], F32)
                        nc.vector.transpose(T2[:], P1[:])
                        t2s[p] = T2

                    if dma_only or p < 1:
                        continue
                    pp = p - 1
                    T2 = t2s.pop(pp)
                    # MM2 (fp32 lhsT, fp32 rhs — no convert needed):
                    # P2[32a+u, 32q+v] = Y_i[v,u]
                    P2 = pbpool.tile([128, FREE], F32, tag="psB")
                    for b in range(2):
                        nc.tensor.matmul(
                            P2[:, 512 * b: 512 * (b + 1)],
                            bd_f32[:, :],
                            T2[:, 512 * b: 512 * (b + 1)],
                            start=True, stop=True,
                        )

                    # T_post (DVE flat): S[32a+v, 32q+u] = Y_i[v,u]
                    S = spool.tile([128, FREE], F32)
                    nc.vector.transpose(S[:], P2[:])
                    souts.append(S)

                # ---- phase 2: gated strided stores, interleaved ----------
                # selfcopy(p) is an ACT engine op emitted after all tokens:
                # in-order ACT execution puts it after the last load; its
                # write to S[0,0] (value-preserving) gives store(p) a RAW
                # dep. Interleaved emission so each store fires as soon as
                # its T_post is done (but never before the last load).
                for p in range(NPACK):
                    base = p * PACK
                    nc.scalar.copy(souts[p][0:1, 0:1], souts[p][0:1, 0:1])
                    nc.scalar.dma_start(
                        y_ext[base: base + PACK].rearrange(
                            "(q a) h w -> a h q w", a=4),
                        souts[p].rearrange("p (q u) -> p q u", u=N),
                    )

    return nc


def _run(x_flat, trace=False, mm_dtype_name="bfloat16"):
    from concourse.bass_utils import run_bass_kernel_spmd

    _install_tilefix()
    nc = _build_program(mm_dtype_name)

    pconst = _bd_matrix()

    core_ids = list(range(NCORES))
    in_maps = [
        {"x": np.ascontiguousarray(x_flat[i * PER_CORE:(i + 1) * PER_CORE]),
         "pconst": pconst}
        for i in core_ids
    ]
    bkr = run_bass_kernel_spmd(nc, in_maps, core_ids, trace=trace)
    out = np.concatenate([bkr.results[i]["y"] for i in core_ids], axis=0)
    return out, bkr


def kernel(x):
    x = np.asarray(x, dtype=np.float32)
    x_flat = x.reshape(IMG_TOTAL, N, N)
    out, _ = _run(x_flat, trace=False)
    return out.reshape(x.shape).astype(np.float32)


# revision 13
# speedup vs baseline: 1.0621x; 1.0621x over previous
"""Trainium2 Bass kernel for nn_Lowpass: 2D DCT -> keep 15x15 low-freq block -> 2D IDCT.

The op collapses to out[b,c] = P @ x[b,c] @ P^T with P = Di[:, :15] @ D[:15, :]
(a fixed 32x32 projection), data-parallel over 8 NeuronCores (3072 images each).

Measured-cost-driven design ("two-phase"). Key facts measured on this HW:
  - strided DMA (128B elements) runs ~full rate ALONE (~1.1us / 0.5MB) but
    any two concurrent DMA streams where one is strided collapse to ~3-4x
    worse aggregate. Contiguous+contiguous coexist fine.
  - DVE stream transpose: flat src ~1.1us per [128,1024]; strided src ~2.3us.
    fp32r matmul is 4 PE cycles/row on real HW (no faster than fp32);
    bf16 with FLAT rhs is fast; any strided matmul rhs is ~2x.
So: let the DMA absorb the first shuffle (strided load puts h on partitions)
and the last shuffle (strided store from T_post's natural layout), keeping the
load window and store window DISJOINT in time (outputs parked in SBUF, 12MB).
DVE only does the two mandatory flat PSUM-evicting transposes per pack.

Per 128-image pack (image i = 4q+a, partitions grouped a in [0,4)):
  phase 1 (loads + compute, stores gated off):
    load (sync, strided):  L2[32a+h, 32q+w] = X_i[h,w]
    conv1 (ACT):           L2b = bf16(L2)
    MM1  (PE, lhsT=BD(P^T) bf16, flat rhs):  P1[32a+v, 32q+w] = (P X_i)[v,w]
    T_mid (DVE flat):      T2[32a+w, 32q+v]
    conv2 (Pool):          T2b = bf16(T2)
    MM2  (PE, flat rhs):   P2[32a+u, 32q+v] = Y_i[v,u]
    T_post (DVE flat):     S[32a+v, 32q+u] = Y_i[v,u]   (S persistent, 24 tiles)
  gate: ACT executes its stream in order; 1-element self-copies on each S tile
    are emitted after all conv1(k) [conv1(k) waits load k], so each store's
    RAW dep on its self-copy delays every store past the LAST load.
  phase 2: strided stores (scalar) mirror the load pattern.
"""

import numpy as np

N = 32
FRE = 15
NCORES = 8
IMG_TOTAL = 8192 * 3          # 24576 images of 32x32
PER_CORE = IMG_TOTAL // NCORES  # 3072
PACK = 128                    # images per pipeline iteration (0.5 MB)
NPACK = PER_CORE // PACK      # 24


def _install_tilefix():
    """This container's walrus build rejects instructions carrying >1 sem wait
    ("Too many sync wait commands" in setupSyncWait). Tile attaches all of an
    instruction's required waits to the instruction itself. Split: for any
    instruction with N>1 waits, hoist N-1 of them onto fresh same-engine nop
    instructions placed immediately before it (same blocking semantics, one
    wait per instruction). Same treatment for the kernel-tail drain."""
    from concourse import mybir, tile
    from concourse.vector_clock import ScopedClock, VectorClock

    if getattr(tile.TileContext, "_tilefix_installed", False):
        return

    orig_lower = tile.TileContext._lower_ordered_insts

    def _lower_split(self, postordered_blocks):
        nc = self.nc
        for insts in postordered_blocks.values():
            new = []
            for inst in insts:
                si = getattr(inst, "sync_info", None)
                ow = list(si.on_wait) if si is not None and si.on_wait else []
                if len(ow) > 1:
                    for w in ow[:-1]:
                        nop = mybir.InstNoOp(
                            name=nc.get_next_instruction_name(), ins=[], outs=[])
                        nop.engine = inst.engine
                        nop.sync_info = mybir.SyncInfo(
                            on_wait=[w], on_update=[])
                        new.append(nop)
                    inst.sync_info = mybir.SyncInfo(
                        on_wait=[ow[-1]], on_update=list(si.on_update))
                new.append(inst)
            insts[:] = new
        return orig_lower(self, postordered_blocks)

    def _drain_and_barrier_split(self, tick_clock, wait_clock):
        nc = self.nc
        gc = tick_clock.global_clock
        n = len(gc)
        for proc in range(n):
            t = gc[proc]
            if t <= 0:
                continue
            vec = [0] * n
            vec[proc] = t
            nop_inst = nc.sync.nop()
            wait_clock.add_sem_waits(
                nop_inst.ins, ScopedClock({None: VectorClock(vec)})
            )
        nc.sync.drain()
        nc.all_engine_barrier()
        assert self.sems is not None
        popped = nc._tile_sem_poison_stack.pop()
        assert popped is self._sem_poison
        nc.clear_and_free_semaphores(list(self.sems.allocated().values()))
        nc.all_engine_barrier()

    tile.TileContext._lower_ordered_insts = _lower_split
    tile.TileContext._drain_and_barrier = _drain_and_barrier_split
    tile.TileContext._tilefix_installed = True

    # NTFF profiling hooks don't exist in this container; make trace=True
    # degrade gracefully inside run_bass_kernel_spmd.
    import sys as _sys
    import types as _types
    if "antenv.axon_hooks" not in _sys.modules:
        m = _types.ModuleType("antenv.axon_hooks")
        m.get_axon_ntff_profile_hook = lambda: None
        _sys.modules["antenv.axon_hooks"] = m


def _p_matrix():
    i = np.arange(N)
    D = 2.0 * np.cos(np.pi * (2 * i[None, :] + 1) * i[:, None] / (2 * N))
    Di = np.linalg.inv(D)
    P = Di[:, :FRE] @ D[:FRE, :]        # float64 [32, 32]
    return P


def _bd_matrix():
    # lhsT = block-diag(P^T): BD[32A+h, 32A+v] = P[v, h]
    P = _p_matrix()
    return np.kron(np.eye(4), P.T).astype(np.float32)  # [128, 128]


def _build_program(mm_dtype_name="bfloat16", loop_reps=1, dma_only=False):
    from concourse import bass, tile
    from concourse import mybir

    F32 = mybir.dt.float32
    MMDT = getattr(mybir.dt, mm_dtype_name)
    use_conv = mm_dtype_name != "float32" and not dma_only
    FREE = PACK * 8             # 1024 free elems per [128, FREE] tile

    nc = bass.Bass("TRN2", target_bir_lowering=False, debug=False,
                   num_devices=NCORES)
    x_ext = nc.dram_tensor("x", [PER_CORE, N, N], F32, kind="ExternalInput").ap()
    p_ext = nc.dram_tensor("pconst", [128, 128], F32, kind="ExternalInput").ap()
    y_ext = nc.dram_tensor("y", [PER_CORE, N, N], F32, kind="ExternalOutput").ap()

    with tile.TileContext(nc) as tc:
        with tc.tile_pool(name="const", bufs=1) as cpool, \
             tc.tile_pool(name="xin", bufs=NPACK if dma_only else 4) as xpool, \
             tc.tile_pool(name="xb", bufs=NPACK) as xbpool, \
             tc.tile_pool(name="tok", bufs=2) as tokpool, \
             tc.tile_pool(name="t2", bufs=4) as t2pool, \
             tc.tile_pool(name="sout", bufs=NPACK) as spool, \
             tc.tile_pool(name="psA", bufs=2, space="PSUM") as papool, \
             tc.tile_pool(name="psB", bufs=2, space="PSUM") as pbpool:

            bd_f32 = cpool.tile([128, 128], F32)
            nc.sync.dma_start(bd_f32[:], p_ext[:])
            if use_conv:
                bd_mm = cpool.tile([128, 128], MMDT)
                nc.scalar.copy(bd_mm[:], bd_f32[:])
            else:
                bd_mm = bd_f32

            for rep in range(loop_reps):
                souts = []
                t2s = {}
                tok = tokpool.tile([1, NPACK], F32)
                # -------- phase 1: loads + compute, 2-pack skew -----------
                # Engines execute their streams in order, so a stalled
                # instruction blocks ready ones behind it. With MM2/T_post
                # delayed 2 packs, every MM2(p-2) input (T_mid(p-2)) is
                # long done: PE never stalls (so it also ramps to full
                # p-state) and DVE never waits on PE.
                SKEW = 2
                for p in range(NPACK + SKEW):
                    if p < NPACK:
                        base = p * PACK
                        # strided load (128B elements): h on partitions.
                        # i = 4q + a; L2[32a+h, 32q+w] = X_i[h,w]
                        L2 = xpool.tile([128, FREE], F32)
                        nc.sync.dma_start(
                            L2.rearrange("p (q w) -> p q w", w=N),
                            x_ext[base: base + PACK].rearrange(
                                "(q a) h w -> a h q w", a=4),
                        )
                        # gate token: a cheap ACT op consuming this load.
                        # ACT runs its stream in order, so anything emitted
                        # on ACT after token(NPACK-1) executes after ALL
                        # loads completed.
                        nc.scalar.copy(tok[0:1, p:p + 1], L2[0:1, 0:1])
                        if dma_only:
                            souts.append(L2)
                            continue

                        if use_conv:
                            # conv1 (ACT): bf16 copy consumed by MM1. Deep
                            # pool (bufs=NPACK) so DVE-paced MM1 consumption
                            # never back-throttles conv1 -> loads.
                            L2m = xbpool.tile([128, FREE], MMDT)
                            nc.scalar.copy(L2m[:], L2[:])
                        else:
                            L2m = L2

                        # MM1 (bf16): P1[32a+v, 32q+w] = (P X_i)[v,w]
                        P1 = papool.tile([128, FREE], F32, tag="psA")
                        for b in range(2):
                            nc.tensor.matmul(
                                P1[:, 512 * b: 512 * (b + 1)],
                                bd_mm[:, :],
                                L2m[:, 512 * b: 512 * (b + 1)],
                                start=True, stop=True,
                            )

                        # T_mid (DVE flat): T2[32a+w, 32q+v] = (P X_i)[v,w]
                        T2 = t2pool.tile([128, FREE], F32)
                        nc.vector.transpose(T2[:], P1[:])
                        t2s[p] = T2

                    if dma_only or p < SKEW:
                        continue
                    pp = p - SKEW
                    T2 = t2s.pop(pp)
                    # MM2 (fp32 lhsT, fp32 rhs — no convert needed):
                    # P2[32a+u, 32q+v] = Y_i[v,u]
                    P2 = pbpool.tile([128, FREE], F32, tag="psB")
                    for b in range(2):
                        nc.tensor.matmul(
                            P2[:, 512 * b: 512 * (b + 1)],
                            bd_f32[:, :],
                            T2[:, 512 * b: 512 * (b + 1)],
                            start=True, stop=True,
                        )

                    # T_post (DVE flat): S[32a+v, 32q+u] = Y_i[v,u]
                    S = spool.tile([128, FREE], F32)
                    nc.vector.transpose(S[:], P2[:])
                    souts.append(S)

                # ---- phase 2: gated strided stores, interleaved ----------
                # selfcopy(p) is an ACT engine op emitted after all tokens:
                # in-order ACT execution puts it after the last load; its
                # write to S[0,0] (value-preserving) gives store(p) a RAW
                # dep. Interleaved emission so each store fires as soon as
                # its T_post is done (but never before the last load).
                for p in range(NPACK):
                    base = p * PACK
                    nc.scalar.copy(souts[p][0:1, 0:1], souts[p][0:1, 0:1])
                    nc.scalar.dma_start(
                        y_ext[base: base + PACK].rearrange(
                            "(q a) h w -> a h q w", a=4),
                        souts[p].rearrange("p (q u) -> p q u", u=N),
                    )

    return nc


def _run(x_flat, trace=False, mm_dtype_name="bfloat16"):
    from concourse.bass_utils import run_bass_kernel_spmd

    _install_tilefix()
    nc = _build_program(mm_dtype_name)

    pconst = _bd_matrix()

    core_ids = list(range(NCORES))
    in_maps = [
        {"x": np.ascontiguousarray(x_flat[i * PER_CORE:(i + 1) * PER_CORE]),
         "pconst": pconst}
        for i in core_ids
    ]
    bkr = run_bass_kernel_spmd(nc, in_maps, core_ids, trace=trace)
    out = np.concatenate([bkr.results[i]["y"] for i in core_ids], axis=0)
    return out, bkr


def kernel(x):
    x = np.asarray(x, dtype=np.float32)
    x_flat = x.reshape(IMG_TOTAL, N, N)
    out, _ = _run(x_flat, trace=False)
    return out.reshape(x.shape).astype(np.float32)


# revision 14
# speedup vs baseline: 1.0764x; 1.0134x over previous
"""Trainium2 Bass kernel for nn_Lowpass: 2D DCT -> keep 15x15 low-freq block -> 2D IDCT.

The op collapses to out[b,c] = P @ x[b,c] @ P^T with P = Di[:, :15] @ D[:15, :]
(a fixed 32x32 projection), data-parallel over 8 NeuronCores (3072 images each).

Measured-cost-driven design ("two-phase"). Key facts measured on this HW:
  - strided DMA (128B elements) runs ~full rate ALONE (~1.1us / 0.5MB) but
    any two concurrent DMA streams where one is strided collapse to ~3-4x
    worse aggregate. Contiguous+contiguous coexist fine.
  - DVE stream transpose: flat src ~1.1us per [128,1024]; strided src ~2.3us.
    fp32r matmul is 4 PE cycles/row on real HW (no faster than fp32);
    bf16 with FLAT rhs is fast; any strided matmul rhs is ~2x.
So: let the DMA absorb the first shuffle (strided load puts h on partitions)
and the last shuffle (strided store from T_post's natural layout), keeping the
load window and store window DISJOINT in time (outputs parked in SBUF, 12MB).
DVE only does the two mandatory flat PSUM-evicting transposes per pack.

Per 128-image pack (image i = 4q+a, partitions grouped a in [0,4)):
  phase 1 (loads + compute, stores gated off):
    load (sync, strided):  L2[32a+h, 32q+w] = X_i[h,w]
    conv1 (ACT):           L2b = bf16(L2)
    MM1  (PE, lhsT=BD(P^T) bf16, flat rhs):  P1[32a+v, 32q+w] = (P X_i)[v,w]
    T_mid (DVE flat):      T2[32a+w, 32q+v]
    conv2 (Pool):          T2b = bf16(T2)
    MM2  (PE, flat rhs):   P2[32a+u, 32q+v] = Y_i[v,u]
    T_post (DVE flat):     S[32a+v, 32q+u] = Y_i[v,u]   (S persistent, 24 tiles)
  gate: ACT executes its stream in order; 1-element self-copies on each S tile
    are emitted after all conv1(k) [conv1(k) waits load k], so each store's
    RAW dep on its self-copy delays every store past the LAST load.
  phase 2: strided stores (scalar) mirror the load pattern.
"""

import numpy as np

N = 32
FRE = 15
NCORES = 8
IMG_TOTAL = 8192 * 3          # 24576 images of 32x32
PER_CORE = IMG_TOTAL // NCORES  # 3072
PACK = 128                    # images per pipeline iteration (0.5 MB)
NPACK = PER_CORE // PACK      # 24


def _install_tilefix():
    """This container's walrus build rejects instructions carrying >1 sem wait
    ("Too many sync wait commands" in setupSyncWait). Tile attaches all of an
    instruction's required waits to the instruction itself. Split: for any
    instruction with N>1 waits, hoist N-1 of them onto fresh same-engine nop
    instructions placed immediately before it (same blocking semantics, one
    wait per instruction). Same treatment for the kernel-tail drain."""
    from concourse import mybir, tile
    from concourse.vector_clock import ScopedClock, VectorClock

    if getattr(tile.TileContext, "_tilefix_installed", False):
        return

    orig_lower = tile.TileContext._lower_ordered_insts

    def _lower_split(self, postordered_blocks):
        nc = self.nc
        for insts in postordered_blocks.values():
            new = []
            for inst in insts:
                si = getattr(inst, "sync_info", None)
                ow = list(si.on_wait) if si is not None and si.on_wait else []
                if len(ow) > 1:
                    for w in ow[:-1]:
                        nop = mybir.InstNoOp(
                            name=nc.get_next_instruction_name(), ins=[], outs=[])
                        nop.engine = inst.engine
                        nop.sync_info = mybir.SyncInfo(
                            on_wait=[w], on_update=[])
                        new.append(nop)
                    inst.sync_info = mybir.SyncInfo(
                        on_wait=[ow[-1]], on_update=list(si.on_update))
                new.append(inst)
            insts[:] = new
        return orig_lower(self, postordered_blocks)

    def _drain_and_barrier_split(self, tick_clock, wait_clock):
        nc = self.nc
        gc = tick_clock.global_clock
        n = len(gc)
        for proc in range(n):
            t = gc[proc]
            if t <= 0:
                continue
            vec = [0] * n
            vec[proc] = t
            nop_inst = nc.sync.nop()
            wait_clock.add_sem_waits(
                nop_inst.ins, ScopedClock({None: VectorClock(vec)})
            )
        nc.sync.drain()
        nc.all_engine_barrier()
        assert self.sems is not None
        popped = nc._tile_sem_poison_stack.pop()
        assert popped is self._sem_poison
        nc.clear_and_free_semaphores(list(self.sems.allocated().values()))
        nc.all_engine_barrier()

    tile.TileContext._lower_ordered_insts = _lower_split
    tile.TileContext._drain_and_barrier = _drain_and_barrier_split
    tile.TileContext._tilefix_installed = True

    # NTFF profiling hooks don't exist in this container; make trace=True
    # degrade gracefully inside run_bass_kernel_spmd.
    import sys as _sys
    import types as _types
    if "antenv.axon_hooks" not in _sys.modules:
        m = _types.ModuleType("antenv.axon_hooks")
        m.get_axon_ntff_profile_hook = lambda: None
        _sys.modules["antenv.axon_hooks"] = m


def _p_matrix():
    i = np.arange(N)
    D = 2.0 * np.cos(np.pi * (2 * i[None, :] + 1) * i[:, None] / (2 * N))
    Di = np.linalg.inv(D)
    P = Di[:, :FRE] @ D[:FRE, :]        # float64 [32, 32]
    return P


def _bd_matrix():
    # lhsT = block-diag(P^T): BD[32A+h, 32A+v] = P[v, h]
    P = _p_matrix()
    return np.kron(np.eye(4), P.T).astype(np.float32)  # [128, 128]


def _build_program(mm_dtype_name="bfloat16", loop_reps=1, dma_only=False):
    from concourse import bass, tile
    from concourse import mybir

    F32 = mybir.dt.float32
    MMDT = getattr(mybir.dt, mm_dtype_name)
    use_conv = mm_dtype_name != "float32" and not dma_only
    FREE = PACK * 8             # 1024 free elems per [128, FREE] tile

    nc = bass.Bass("TRN2", target_bir_lowering=False, debug=False,
                   num_devices=NCORES)
    x_ext = nc.dram_tensor("x", [PER_CORE, N, N], F32, kind="ExternalInput").ap()
    p_ext = nc.dram_tensor("pconst", [128, 128], F32, kind="ExternalInput").ap()
    y_ext = nc.dram_tensor("y", [PER_CORE, N, N], F32, kind="ExternalOutput").ap()

    with tile.TileContext(nc) as tc:
        with tc.tile_pool(name="const", bufs=1) as cpool, \
             tc.tile_pool(name="xin", bufs=NPACK if dma_only else 4) as xpool, \
             tc.tile_pool(name="xb", bufs=NPACK) as xbpool, \
             tc.tile_pool(name="tok", bufs=2) as tokpool, \
             tc.tile_pool(name="t2", bufs=4) as t2pool, \
             tc.tile_pool(name="sout", bufs=NPACK) as spool, \
             tc.tile_pool(name="psA", bufs=2, space="PSUM") as papool, \
             tc.tile_pool(name="psB", bufs=2, space="PSUM") as pbpool:

            bd_f32 = cpool.tile([128, 128], F32)
            nc.sync.dma_start(bd_f32[:], p_ext[:])
            if use_conv:
                bd_mm = cpool.tile([128, 128], MMDT)
                nc.scalar.copy(bd_mm[:], bd_f32[:])
            else:
                bd_mm = bd_f32

            for rep in range(loop_reps):
                souts = []
                t2s = {}
                tok = tokpool.tile([1, NPACK], F32)
                # -------- phase 1: loads + compute, 2-pack skew -----------
                # Engines execute their streams in order, so a stalled
                # instruction blocks ready ones behind it. With MM2/T_post
                # delayed 2 packs, every MM2(p-2) input (T_mid(p-2)) is
                # long done: PE never stalls (so it also ramps to full
                # p-state) and DVE never waits on PE.
                SKEW = 0
                for p in range(NPACK + SKEW):
                    if p < NPACK:
                        base = p * PACK
                        # strided load (128B elements): h on partitions.
                        # i = 4q + a; L2[32a+h, 32q+w] = X_i[h,w]
                        L2 = xpool.tile([128, FREE], F32)
                        nc.sync.dma_start(
                            L2.rearrange("p (q w) -> p q w", w=N),
                            x_ext[base: base + PACK].rearrange(
                                "(q a) h w -> a h q w", a=4),
                        )
                        # gate token: a cheap ACT op consuming this load.
                        # ACT runs its stream in order, so anything emitted
                        # on ACT after token(NPACK-1) executes after ALL
                        # loads completed.
                        nc.scalar.copy(tok[0:1, p:p + 1], L2[0:1, 0:1])
                        if dma_only:
                            souts.append(L2)
                            continue

                        if use_conv:
                            # conv1 (ACT): bf16 copy consumed by MM1. Deep
                            # pool (bufs=NPACK) so DVE-paced MM1 consumption
                            # never back-throttles conv1 -> loads.
                            L2m = xbpool.tile([128, FREE], MMDT)
                            nc.scalar.copy(L2m[:], L2[:])
                        else:
                            L2m = L2

                        # MM1 (bf16): P1[32a+v, 32q+w] = (P X_i)[v,w]
                        P1 = papool.tile([128, FREE], F32, tag="psA")
                        for b in range(2):
                            nc.tensor.matmul(
                                P1[:, 512 * b: 512 * (b + 1)],
                                bd_mm[:, :],
                                L2m[:, 512 * b: 512 * (b + 1)],
                                start=True, stop=True,
                            )

                        # T_mid (DVE flat): T2[32a+w, 32q+v] = (P X_i)[v,w]
                        T2 = t2pool.tile([128, FREE], F32)
                        nc.vector.transpose(T2[:], P1[:])
                        t2s[p] = T2

                    if dma_only or p < SKEW:
                        continue
                    pp = p - SKEW
                    T2 = t2s.pop(pp)
                    # MM2 (fp32 lhsT, fp32 rhs — no convert needed):
                    # P2[32a+u, 32q+v] = Y_i[v,u]
                    P2 = pbpool.tile([128, FREE], F32, tag="psB")
                    for b in range(2):
                        nc.tensor.matmul(
                            P2[:, 512 * b: 512 * (b + 1)],
                            bd_f32[:, :],
                            T2[:, 512 * b: 512 * (b + 1)],
                            start=True, stop=True,
                        )

                    # T_post (DVE flat): S[32a+v, 32q+u] = Y_i[v,u]
                    S = spool.tile([128, FREE], F32)
                    nc.vector.transpose(S[:], P2[:])
                    souts.append(S)

                # ---- phase 2: gated strided stores, interleaved ----------
                # selfcopy(p) is an ACT engine op emitted after all tokens:
                # in-order ACT execution puts it after the last load; its
                # write to S[0,0] (value-preserving) gives store(p) a RAW
                # dep. Interleaved emission so each store fires as soon as
                # its T_post is done (but never before the last load).
                for p in range(NPACK):
                    base = p * PACK
                    nc.scalar.copy(souts[p][0:1, 0:1], souts[p][0:1, 0:1])
                    nc.scalar.dma_start(
                        y_ext[base: base + PACK].rearrange(
                            "(q a) h w -> a h q w", a=4),
                        souts[p].rearrange("p (q u) -> p q u", u=N),
                    )

    return nc


def _run(x_flat, trace=False, mm_dtype_name="bfloat16"):
    from concourse.bass_utils import run_bass_kernel_spmd

    _install_tilefix()
    nc = _build_program(mm_dtype_name)

    pconst = _bd_matrix()

    core_ids = list(range(NCORES))
    in_maps = [
        {"x": np.ascontiguousarray(x_flat[i * PER_CORE:(i + 1) * PER_CORE]),
         "pconst": pconst}
        for i in core_ids
    ]
    bkr = run_bass_kernel_spmd(nc, in_maps, core_ids, trace=trace)
    out = np.concatenate([bkr.results[i]["y"] for i in core_ids], axis=0)
    return out, bkr


def kernel(x):
    x = np.asarray(x, dtype=np.float32)
    x_flat = x.reshape(IMG_TOTAL, N, N)
    out, _ = _run(x_flat, trace=False)
    return out.reshape(x.shape).astype(np.float32)
